# revision 28
# baseline (speedup 1.0000x reference)
"""Trainium2 Bass kernel for the LeNet C3 dense-conv layer.

Computes out = conv2d_valid(x, K, stride 1) + bias where K is the dense
[16, 6, 5, 5] kernel scattered from the sparse per-branch weights
(w3/w4/w6), x is [128, 6, 256, 256] f32, out is [128, 16, 252, 252] f32.

Strategy:
  - Pure data parallelism: 16 images per NeuronCore across 8 cores.
  - The conv is shift-accumulated banded matmuls into PSUM, with the
    input packed by COLUMN PARITY: a block covers 6 output rows; the
    contraction dim stacks the 10 input rows x 6 channels twice
    (60+60 = 120 partitions), the first half holding EVEN input
    columns, the second half ODD columns. Even and odd OUTPUT columns
    accumulate in separate PSUM banks; each needs 3 matmuls at
    u-column offsets 0/+1/+2 whose stationary "walls" route the right
    kernel column kx to the right parity half:
      even out[2u]  : off0 {kx0->even, kx1->odd}, off1 {kx2,kx3}, off2 {kx4}
      odd out[2u+1] : off0 {kx0->odd}, off1 {kx1,kx2}, off2 {kx3,kx4}
    Unlike the previous shifted-copy scheme this reads every input
    byte from HBM exactly once per block (no 2x duplication), while
    keeping the identical matmul shape [K=120, M=96, N=512] and count
    (1008): N=512 covers FOUR images x 128 half-columns. K=120 > 96
    keeps all four PE row-group quarters active (full 1 col/cycle
    stream rate); M=96 avoids fast-weight-load rate loss.
  - The banded lhsT column m = c_out*6 + r holds K_dense[c_out, c_in,
    i-r, kx] at row i*6 + c_in (+60 for the odd-parity half).
  - Each block needs a single 492 KB input DMA ([120, 4100 B] rows,
    striped over all 16 SDMA engines). Per 8-image sub-round, a 4-bank
    PSUM tile (quad0-even, quad0-odd, quad1-even, quad1-odd) is
    evicted by one vector op with the bias add fused, then written by
    one 387 KB output DMA (4 KB descriptors). Matmul seam pollution
    (offsets +1/+2 crossing image boundaries) lands on u>=126, which
    the eviction drops (only u 0..125 = output w = 2u+parity < 252).
  - fp16 operands (~3e-4 rel err; accumulation is fp32 in PSUM).
  - Host packs/unpacks the interleaved layouts (x fp16 cast + parity
    split; output o8[sr, co, h, quad, parity, img, u] -> NCHW).
"""

import numpy as np

# LeNet-5 C3 sparse channel connectivity (from the model definition).
CH3 = np.array([[0, 1, 2], [1, 2, 3], [2, 3, 4], [3, 4, 5], [0, 4, 5], [0, 1, 5]])
CH4 = np.array([[0, 1, 2, 3], [1, 2, 3, 4], [2, 3, 4, 5], [0, 3, 4, 5],
                [0, 1, 4, 5], [0, 1, 2, 5], [0, 1, 3, 4], [1, 2, 4, 5],
                [0, 2, 3, 5]])

B, C, H, W = 128, 6, 256, 256
CO, HO, WO = 16, 252, 252
NCORES = 8
BPC = B // NCORES           # images per core (16)
KH = KW = 5

R = 6                       # output rows per block
HI = R + 4                  # input rows per block (10)
NBLK = HO // R              # 42 blocks
KK = C * HI                 # contraction rows per parity half (60)
MM = CO * R                 # psum partitions (96)
HW = W // 2                 # half-columns per image (128)
UO = WO // 2                # valid output half-columns (126)
TW = BPC * HW               # input tile width: 16 img * 128 (2048)

_STATE = None  # cached Bass module so repeat kernel() calls skip re-tracing


def _dense_kernel(w3, w4, w6):
    k = np.zeros((CO, C, KH, KW), np.float32)
    k[np.arange(6)[:, None], CH3] = w3
    k[6 + np.arange(9)[:, None], CH4] = w4
    k[15] = w6[0]
    return k


def _band(kd, kx):
    """Banded lhsT [KK, MM] for kernel column kx: row i*6 + c_in,
    column c_out*R + r, value kd[c_out, c_in, i-r, kx]."""
    out = np.zeros((KK, MM), np.float32)
    for ci in range(C):
        for i in range(HI):
            for r in range(R):
                ky = i - r
                if 0 <= ky < KH:
                    out[i * C + ci, np.arange(CO) * R + r] = kd[:, ci, ky, kx]
    return out


def _build_module():
    import concourse.bacc as bacc
    import concourse.mybir as mybir
    from concourse.tile import TileContext

    f32 = mybir.dt.float32
    f16 = mybir.dt.float16

    # Bacc (not Bass): its compile() runs generate_event_semaphores(),
    # which splits multi-wait instructions to satisfy the TRN2 1-wait-
    # per-instruction constraint walrus enforces.
    nc = bacc.Bacc(None)
    # Parity-packed per-block input tiles (see module docstring).
    x_d = nc.dram_tensor("x", [NBLK, 2 * KK, TW], f16, kind="ExternalInput")
    # wall: [120, 6*96], six [120, 96] stationaries (even off0/1/2 then
    # odd off0/1/2), each [even-half band; odd-half band].
    wall_d = nc.dram_tensor("wall", [2 * KK, 6 * MM], f16, kind="ExternalInput")
    b1_d = nc.dram_tensor("b1", [MM, 1], f32, kind="ExternalInput")
    # o8[sr, co, h, quad01, parity, img, u] = out[8*sr+4*q+img, co, h, 2u+parity]
    # (host un-packs). Staged in fp16 to halve the dominant output DMA
    # stream; the host casts back to f32 (~5e-4 relative quantization).
    o_d = nc.dram_tensor("o", [2, CO, HO, 2 * 2 * 4 * UO], f16,
                         kind="ExternalOutput")

    with TileContext(nc) as tc:
        with (
            tc.tile_pool(name="wpool", bufs=1) as wp,
            tc.tile_pool(name="inpool", bufs=3) as ip,
            tc.tile_pool(name="outpool", bufs=6) as op,
            tc.tile_pool(name="pspool", bufs=2, space="PSUM") as pp,
        ):
            # The wall leads the sync queue (first matmul gates on it); b1
            # rides the scalar queue in parallel (not needed until the
            # first eviction).
            wall_t = wp.tile([2 * KK, 6 * MM], f16)
            nc.sync.dma_start(wall_t[:], wall_d[:])
            b1_t = wp.tile([MM, 1], f32)
            nc.scalar.dma_start(b1_t[:], b1_d[:])

            # Prime the constant tiles on their consuming engine classes so
            # steady-state instructions carry few semaphore waits.
            prime_ps = pp.tile([MM, 16, 128], f32, tag="ps")
            nc.tensor.matmul(prime_ps[:, 0, :], wall_t[:, 0:MM],
                             wall_t[:, 0:128], start=True, stop=True)
            prime_b = op.tile([MM, 2], f16, tag="out")
            nc.vector.tensor_scalar_add(prime_b[:, 0:1], b1_t[:], 0.0)
            nc.scalar.activation(prime_b[:, 1:2], b1_t[:],
                                 mybir.ActivationFunctionType.Identity,
                                 bias=b1_t[:, 0:1])

            for g in range(NBLK):
                h0 = R * g
                it = ip.tile([2 * KK, TW], f16, tag="in")
                if g == 0:
                    # Split block 0's load so the first sub-round starts
                    # as soon as its half lands.
                    for c in range(2):
                        nc.sync.dma_start(it[:, 1024 * c:1024 * (c + 1)],
                                          x_d[g, :, 1024 * c:1024 * (c + 1)])
                else:
                    nc.sync.dma_start(it[:, :], x_d[g])

                for sr in range(2):             # 8-image sub-rounds
                    # One 4-bank PSUM tile: (q0-even, q0-odd, q1-e, q1-o).
                    ps = pp.tile([MM, 16, 128], f32, tag="ps")
                    ot = op.tile([MM, 16 * UO], f16, tag="out")
                    for q in range(2):          # 4-image quads
                        b = 1024 * sr + 512 * q
                        quad = it[:, b:b + 512].rearrange(
                            "p (j u) -> p j u", u=128)
                        for par in range(2):    # even / odd output cols
                            # Strided N: stream only the 504 useful
                            # columns (u 0..125 per image), skipping the
                            # seam half-columns entirely.
                            pslice = ps[:, 8 * par + 4 * q:
                                        8 * par + 4 * q + 4, 0:UO]
                            w0 = (3 * par) * MM
                            for d in range(3):  # u-offset 0/+1/+2
                                nc.tensor.matmul(
                                    pslice, wall_t[:, w0 + d * MM:
                                                   w0 + (d + 1) * MM],
                                    quad[:, :, d:d + UO],
                                    start=(d == 0), stop=(d == 2))
                    # Eviction split across the vector and activation
                    # engines (halves the PSUM-free latency), bias fused,
                    # fp16 cast; drops the seam half-cols u >= 126.
                    nc.vector.tensor_scalar_add(
                        ot[:, 0:8 * UO].rearrange("p (j u) -> p j u", j=8),
                        ps[:, 0:8, 0:UO],
                        b1_t[:, 0:1],
                    )
                    nc.scalar.activation(
                        ot[:, 8 * UO:16 * UO]
                        .rearrange("p (j u) -> p j u", j=8),
                        ps[:, 8:16, 0:UO],
                        mybir.ActivationFunctionType.Identity,
                        bias=b1_t[:, 0:1],
                    )

                    if g == NBLK - 1 and sr == 1:
                        # Split the final output DMA so the first half
                        # streams while the second trigger is queued.
                        nc.gpsimd.dma_start(o_d[sr, :, h0:h0 + R, 0:8 * UO],
                                            ot[:, 0:8 * UO])
                        nc.gpsimd.dma_start(o_d[sr, :, h0:h0 + R,
                                                8 * UO:16 * UO],
                                            ot[:, 8 * UO:16 * UO])
                    else:
                        nc.gpsimd.dma_start(o_d[sr, :, h0:h0 + R, :], ot[:])
    nc.compile()
    return nc


def _get_module():
    global _STATE
    if _STATE is None:
        _STATE = _build_module()
    return _STATE


def kernel(x, w3, b3, w4, b4, w6, b6):
    from concourse.bass_utils import run_bass_kernel_spmd

    x = np.asarray(x, np.float32)
    kd = _dense_kernel(np.asarray(w3, np.float32), np.asarray(w4, np.float32),
                       np.asarray(w6, np.float32))
    bias = np.concatenate([np.asarray(b3, np.float32),
                           np.asarray(b4, np.float32),
                           np.asarray(b6, np.float32)])

    zero = np.zeros((KK, MM), np.float32)
    bands = [_band(kd, kx) for kx in range(KW)]
    # Column m-blocks: even outputs off 0/1/2, then odd outputs off 0/1/2.
    wall = np.concatenate([
        np.concatenate([bands[0], bands[2], bands[4], zero, bands[1], bands[3]],
                       axis=1),                       # even-parity K rows
        np.concatenate([bands[1], bands[3], zero, bands[0], bands[2], bands[4]],
                       axis=1),                       # odd-parity K rows
    ], axis=0).astype(np.float16)
    b1 = np.repeat(bias, R).astype(np.float32).reshape(MM, 1)

    nc = _get_module()
    x16 = x.astype(np.float16)
    in_maps = []
    for cr in range(NCORES):
        xs = x16[cr * BPC:(cr + 1) * BPC]
        # rows_p[(h, c), img*128 + u] = x[img, c, h, 2u + p]
        ev = np.ascontiguousarray(
            xs[:, :, :, 0::2].transpose(2, 1, 0, 3)).reshape(H * C, BPC * HW)
        od = np.ascontiguousarray(
            xs[:, :, :, 1::2].transpose(2, 1, 0, 3)).reshape(H * C, BPC * HW)
        xstk = np.zeros((NBLK, 2 * KK, TW), np.float16)
        for g in range(NBLK):
            xstk[g, 0:KK, 0:BPC * HW] = ev[R * C * g: R * C * g + KK]
            xstk[g, KK:2 * KK, 0:BPC * HW] = od[R * C * g: R * C * g + KK]
        in_maps.append({"x": xstk, "wall": wall, "b1": b1})
    res = run_bass_kernel_spmd(nc, in_maps, core_ids=list(range(NCORES)))
    global LAST_RESULT
    LAST_RESULT = res

    out = np.empty((B, CO, HO, WO), np.float32)
    for cr in range(NCORES):
        o8 = res.results[cr]["o"].astype(np.float32).reshape(
            2, CO, HO, 2, 8, UO)               # (sr, co, h, par, img, u)
        # img_global = 8*sr + img ; w = 2u + par
        out[cr * BPC:(cr + 1) * BPC] = (
            o8.transpose(0, 4, 1, 2, 5, 3)     # (sr, img, co, h, u, par)
            .reshape(BPC, CO, HO, WO)
        )
    return out


LAST_RESULT = None


# revision 30
# speedup vs baseline: 1.2790x; 1.2790x over previous
"""Trainium2 Bass kernel for the LeNet C3 dense-conv layer.

Computes out = conv2d_valid(x, K, stride 1) + bias where K is the dense
[16, 6, 5, 5] kernel scattered from the sparse per-branch weights
(w3/w4/w6), x is [128, 6, 256, 256] f32, out is [128, 16, 252, 252] f32.

Strategy:
  - Pure data parallelism: 16 images per NeuronCore across 8 cores.
  - The conv is shift-accumulated banded matmuls into PSUM, with the
    input packed by COLUMN PARITY: a block covers 6 output rows; the
    contraction dim stacks the 10 input rows x 6 channels twice
    (60+60 = 120 partitions), the first half holding EVEN input
    columns, the second half ODD columns. Even and odd OUTPUT columns
    accumulate in separate PSUM banks; each needs 3 matmuls at
    u-column offsets 0/+1/+2 whose stationary "walls" route the right
    kernel column kx to the right parity half:
      even out[2u]  : off0 {kx0->even, kx1->odd}, off1 {kx2,kx3}, off2 {kx4}
      odd out[2u+1] : off0 {kx0->odd}, off1 {kx1,kx2}, off2 {kx3,kx4}
    Unlike the previous shifted-copy scheme this reads every input
    byte from HBM exactly once per block (no 2x duplication), while
    keeping the identical matmul shape [K=120, M=96, N=512] and count
    (1008): N=512 covers FOUR images x 128 half-columns. K=120 > 96
    keeps all four PE row-group quarters active (full 1 col/cycle
    stream rate); M=96 avoids fast-weight-load rate loss.
  - The banded lhsT column m = c_out*6 + r holds K_dense[c_out, c_in,
    i-r, kx] at row i*6 + c_in (+60 for the odd-parity half).
  - Each block needs a single 492 KB input DMA ([120, 4100 B] rows,
    striped over all 16 SDMA engines). Per 8-image sub-round, a 4-bank
    PSUM tile (quad0-even, quad0-odd, quad1-even, quad1-odd) is
    evicted by one vector op with the bias add fused, then written by
    one 387 KB output DMA (4 KB descriptors). Matmul seam pollution
    (offsets +1/+2 crossing image boundaries) lands on u>=126, which
    the eviction drops (only u 0..125 = output w = 2u+parity < 252).
  - fp16 operands (~3e-4 rel err; accumulation is fp32 in PSUM).
  - Host packs/unpacks the interleaved layouts (x fp16 cast + parity
    split; output o8[sr, co, h, quad, parity, img, u] -> NCHW).
"""

import numpy as np

# LeNet-5 C3 sparse channel connectivity (from the model definition).
CH3 = np.array([[0, 1, 2], [1, 2, 3], [2, 3, 4], [3, 4, 5], [0, 4, 5], [0, 1, 5]])
CH4 = np.array([[0, 1, 2, 3], [1, 2, 3, 4], [2, 3, 4, 5], [0, 3, 4, 5],
                [0, 1, 4, 5], [0, 1, 2, 5], [0, 1, 3, 4], [1, 2, 4, 5],
                [0, 2, 3, 5]])

B, C, H, W = 128, 6, 256, 256
CO, HO, WO = 16, 252, 252
NCORES = 8
BPC = B // NCORES           # images per core (16)
KH = KW = 5

R = 6                       # output rows per block
HI = R + 4                  # input rows per block (10)
NBLK = HO // R              # 42 blocks
KK = C * HI                 # contraction rows per parity half (60)
MM = CO * R                 # psum partitions (96)
HW = W // 2                 # half-columns per image (128)
UO = WO // 2                # valid output half-columns (126)
TW = BPC * HW               # input tile width: 16 img * 128 (2048)

_STATE = None  # cached Bass module so repeat kernel() calls skip re-tracing


def _dense_kernel(w3, w4, w6):
    k = np.zeros((CO, C, KH, KW), np.float32)
    k[np.arange(6)[:, None], CH3] = w3
    k[6 + np.arange(9)[:, None], CH4] = w4
    k[15] = w6[0]
    return k


def _band(kd, kx):
    """Banded lhsT [KK, MM] for kernel column kx: row i*6 + c_in,
    column c_out*R + r, value kd[c_out, c_in, i-r, kx]."""
    out = np.zeros((KK, MM), np.float32)
    for ci in range(C):
        for i in range(HI):
            for r in range(R):
                ky = i - r
                if 0 <= ky < KH:
                    out[i * C + ci, np.arange(CO) * R + r] = kd[:, ci, ky, kx]
    return out


def _build_module():
    import concourse.bacc as bacc
    import concourse.mybir as mybir
    from concourse.tile import TileContext

    f32 = mybir.dt.float32
    f16 = mybir.dt.float16

    # Bacc (not Bass): its compile() runs generate_event_semaphores(),
    # which splits multi-wait instructions to satisfy the TRN2 1-wait-
    # per-instruction constraint walrus enforces.
    nc = bacc.Bacc(None)
    # Parity-packed per-block input tiles (see module docstring).
    x_d = nc.dram_tensor("x", [NBLK, 2 * KK, TW], f16, kind="ExternalInput")
    # wall: [120, 6*96], six [120, 96] stationaries (even off0/1/2 then
    # odd off0/1/2), each [even-half band; odd-half band].
    wall_d = nc.dram_tensor("wall", [2 * KK, 6 * MM], f16, kind="ExternalInput")
    b1_d = nc.dram_tensor("b1", [MM, 1], f32, kind="ExternalInput")
    # o8[sr, co, h, quad01, parity, img, u] = out[8*sr+4*q+img, co, h, 2u+parity]
    # (host un-packs). Staged in fp16 to halve the dominant output DMA
    # stream; the host casts back to f32 (~5e-4 relative quantization).
    o_d = nc.dram_tensor("o", [2, CO, HO, 2 * 2 * 4 * UO], f16,
                         kind="ExternalOutput")

    with TileContext(nc) as tc:
        with (
            tc.tile_pool(name="wpool", bufs=1) as wp,
            tc.tile_pool(name="inpool", bufs=3) as ip,
            tc.tile_pool(name="outpool", bufs=6) as op,
            tc.tile_pool(name="pspool", bufs=2, space="PSUM") as pp,
        ):
            # The wall leads the sync queue (first matmul gates on it); b1
            # rides the scalar queue in parallel (not needed until the
            # first eviction).
            wall_t = wp.tile([2 * KK, 6 * MM], f16)
            nc.sync.dma_start(wall_t[:], wall_d[:])
            b1_t = wp.tile([MM, 1], f32)
            nc.scalar.dma_start(b1_t[:], b1_d[:])

            # Prime the constant tiles on their consuming engine classes so
            # steady-state instructions carry few semaphore waits.
            prime_ps = pp.tile([MM, 16, 128], f32, tag="ps")
            nc.tensor.matmul(prime_ps[:, 0, :], wall_t[:, 0:MM],
                             wall_t[:, 0:128], start=True, stop=True)
            prime_b = op.tile([MM, 1], f16, tag="out")
            nc.vector.tensor_scalar_add(prime_b[:], b1_t[:], 0.0)

            for g in range(NBLK):
                h0 = R * g
                it = ip.tile([2 * KK, TW], f16, tag="in")
                if g == 0:
                    # Split block 0's load so the first sub-round starts
                    # as soon as its half lands.
                    for c in range(2):
                        nc.sync.dma_start(it[:, 1024 * c:1024 * (c + 1)],
                                          x_d[g, :, 1024 * c:1024 * (c + 1)])
                else:
                    nc.sync.dma_start(it[:, :], x_d[g])

                for sr in range(2):             # 8-image sub-rounds
                    # One 4-bank PSUM tile: (q0-even, q0-odd, q1-e, q1-o).
                    ps = pp.tile([MM, 16, 128], f32, tag="ps")
                    ot = op.tile([MM, 16 * UO], f16, tag="out")
                    for q in range(2):          # 4-image quads
                        b = 1024 * sr + 512 * q
                        quad = it[:, b:b + 512].rearrange(
                            "p (j u) -> p j u", u=128)
                        for par in range(2):    # even / odd output cols
                            # Strided N: stream only the 504 useful
                            # columns (u 0..125 per image), skipping the
                            # seam half-columns entirely.
                            pslice = ps[:, 8 * par + 4 * q:
                                        8 * par + 4 * q + 4, 0:UO]
                            w0 = (3 * par) * MM
                            for d in range(3):  # u-offset 0/+1/+2
                                nc.tensor.matmul(
                                    pslice, wall_t[:, w0 + d * MM:
                                                   w0 + (d + 1) * MM],
                                    quad[:, :, d:d + UO],
                                    start=(d == 0), stop=(d == 2))
                    # Single eviction per sub-round, bias fused, fp16
                    # cast; drops the seam half-columns u >= 126.
                    nc.vector.tensor_scalar_add(
                        ot[:].rearrange("p (j u) -> p j u", j=16),
                        ps[:, :, 0:UO],
                        b1_t[:, 0:1],
                    )

                    if g == NBLK - 1 and sr == 1:
                        # Split the final output DMA so the first half
                        # streams while the second trigger is queued.
                        nc.scalar.dma_start(o_d[sr, :, h0:h0 + R, 0:8 * UO],
                                            ot[:, 0:8 * UO])
                        nc.scalar.dma_start(o_d[sr, :, h0:h0 + R,
                                                8 * UO:16 * UO],
                                            ot[:, 8 * UO:16 * UO])
                    else:
                        nc.scalar.dma_start(o_d[sr, :, h0:h0 + R, :], ot[:])
    nc.compile()
    return nc


def _get_module():
    global _STATE
    if _STATE is None:
        _STATE = _build_module()
    return _STATE


def kernel(x, w3, b3, w4, b4, w6, b6):
    from concourse.bass_utils import run_bass_kernel_spmd

    x = np.asarray(x, np.float32)
    kd = _dense_kernel(np.asarray(w3, np.float32), np.asarray(w4, np.float32),
                       np.asarray(w6, np.float32))
    bias = np.concatenate([np.asarray(b3, np.float32),
                           np.asarray(b4, np.float32),
                           np.asarray(b6, np.float32)])

    zero = np.zeros((KK, MM), np.float32)
    bands = [_band(kd, kx) for kx in range(KW)]
    # Column m-blocks: even outputs off 0/1/2, then odd outputs off 0/1/2.
    wall = np.concatenate([
        np.concatenate([bands[0], bands[2], bands[4], zero, bands[1], bands[3]],
                       axis=1),                       # even-parity K rows
        np.concatenate([bands[1], bands[3], zero, bands[0], bands[2], bands[4]],
                       axis=1),                       # odd-parity K rows
    ], axis=0).astype(np.float16)
    b1 = np.repeat(bias, R).astype(np.float32).reshape(MM, 1)

    nc = _get_module()
    x16 = x.astype(np.float16)
    in_maps = []
    for cr in range(NCORES):
        xs = x16[cr * BPC:(cr + 1) * BPC]
        # rows_p[(h, c), img*128 + u] = x[img, c, h, 2u + p]
        ev = np.ascontiguousarray(
            xs[:, :, :, 0::2].transpose(2, 1, 0, 3)).reshape(H * C, BPC * HW)
        od = np.ascontiguousarray(
            xs[:, :, :, 1::2].transpose(2, 1, 0, 3)).reshape(H * C, BPC * HW)
        xstk = np.zeros((NBLK, 2 * KK, TW), np.float16)
        for g in range(NBLK):
            xstk[g, 0:KK, 0:BPC * HW] = ev[R * C * g: R * C * g + KK]
            xstk[g, KK:2 * KK, 0:BPC * HW] = od[R * C * g: R * C * g + KK]
        in_maps.append({"x": xstk, "wall": wall, "b1": b1})
    res = run_bass_kernel_spmd(nc, in_maps, core_ids=list(range(NCORES)))
    global LAST_RESULT
    LAST_RESULT = res

    out = np.empty((B, CO, HO, WO), np.float32)
    for cr in range(NCORES):
        o8 = res.results[cr]["o"].astype(np.float32).reshape(
            2, CO, HO, 2, 8, UO)               # (sr, co, h, par, img, u)
        # img_global = 8*sr + img ; w = 2u + par
        out[cr * BPC:(cr + 1) * BPC] = (
            o8.transpose(0, 4, 1, 2, 5, 3)     # (sr, img, co, h, u, par)
            .reshape(BPC, CO, HO, WO)
        )
    return out


LAST_RESULT = None
